# revision 24
# baseline (speedup 1.0000x reference)
"""AutoCorrelation (Autoformer-style) Bass kernel for one TRN2 chip (8 NeuronCores).

Math: the reference computes, per (b, h):
    corr = irfft(rfft(q, axis=-1) * conj(rfft(k, axis=-1)), n=L)   # [L, L]
    weights = softmax(corr - mean_h(corr), axis=-1)
    Vt = v @ weights                                                # [d, L]
The rfft runs over the d=64 channel axis and the irfft zero-pads 33 bins to
L=2048, so corr[s, :] is a rank-<=66 function of t; the DC term is constant
over t and cancels in softmax.  Collapsing the spectral products
(re*re + im*im -> cos row, im*re - re*im -> sin row) leaves 64 coefficient
rows: the logits are an exact K=64 matmul against a fixed cos/sin basis and
no [L, L] tensor ever exists in DRAM.

Sharding: head h -> core h (both batches per core).  Only the head-mean of
the 64 x 2048 coefficient matrix couples cores.  Default mode (SPLIT=True)
runs two NEFFs: phase A computes coefficients (~40 us), the host sums the
8 cores' 0.5 MB outputs, and phase B (~90 us) does softmax + aggregation —
this is much faster than an on-device AllReduce, which costs 55-60 us of
mostly-fixed latency on this platform (SPLIT=False keeps everything on
device in one NEFF with column-halved AllReduces, ~170 us).

Phase B details: K=64 logits matmuls are row-packed (two concurrent 64-row
PE tiles via base_partition 0/64 of duplicated coefficient/basis tensors);
the delay-aggregation matmuls are column-packed (Vt stored [128, 1024]:
partitions 0-63 hold t 0:1024, partitions 64-127 hold t 1024:2048).  The
softmax exp splits between ScalarE (table exp) and VectorE (custom DVE op
EXP8_ANT: exp(x) ~= (c0 + x(c1 + x c2))^8, valid since logits are bounded
by ~1.5), both with fused free-dim accumulation for the denominator; the
per-row 1/sum folds into the tiny v-tile instead of the weight tile.
"""
import sys
from operator import add as _op_add

sys.path.insert(0, "/opt/trn_rl_repo")

import numpy as np
import ml_dtypes

from concourse import bass, bacc, mybir, tile
from concourse import dve_ops
from concourse.dve_spec import Spec, Src0, C0, C1, C2, Zero, sq, lower
from concourse.dve_uop import DveOpSpec
from concourse.bass_utils import run_bass_kernel_spmd

B, L, E, H, D = 2, 2048, 512, 8, 64
NF = 32          # frequencies 1..32 of the 64-point rfft (DC dropped)
NCOMP = 4 * NF   # 128 raw product rows
NCC = 2 * NF     # 64 compressed coefficient rows (cos, sin)
NCORES = 8
SC = L // 128    # 16 s-chunks of 128 rows
BF16 = mybir.dt.bfloat16
F32 = mybir.dt.float32

# minimax quadratic p(z) for e^z on z = x/8, |x| <= 1.68; exp(x) ~= p(x)^8
EXP_C = (0.99970171, 0.12580122, 0.00795605)

TRACE = False
SPLIT = True
LAST_RESULT = None
LAST_RESULT_A = None

_COMPILED = None
_EXP_OP = None


def _register_exp_op():
    global _EXP_OP
    if _EXP_OP is not None:
        return _EXP_OP
    for o in dve_ops.OPS:
        if o.name == "EXP8_ANT":
            _EXP_OP = o
            return o

    body = sq(sq(sq(C0 + Src0 * (C1 + Src0 * C2))))

    def _ref(in0, in1, c0, c1, c2):
        x = in0.astype(np.float32)
        b = (((c0 + x * (c1 + x * c2)) ** 8)).astype(np.float32)
        return b, b.reshape(b.shape[0], -1).sum(axis=-1, keepdims=True)

    spec = Spec(body=body, accum=_op_add, accum_init=Zero, reference=_ref)
    opcode = dve_ops._CUSTOM_DVE_ROW_BASE + len(dve_ops.OPS)
    dve_ops._SUB_OPCODE_FOR_NAME["EXP8_ANT"] = opcode
    shas = {}
    for ver in ("v3", "v4"):
        shas[ver] = DveOpSpec(
            name="EXP8_ANT", opcode=opcode, uops=lower(spec, ver=ver), rd1_en=False
        ).sha(ver)
    op = dve_ops.DveOp("EXP8_ANT", spec, subdim=False, uops_sha=shas)
    dve_ops.OPS.append(op)
    dve_ops.CUSTOM_DVE_SPECS[op.name] = spec
    _EXP_OP = op
    return op


def _constants():
    c = np.arange(D)
    f = np.arange(1, NF + 1)
    ang = 2 * np.pi * np.outer(c, f) / D
    fcos = np.cos(ang)       # Re X_f   = sum_c q_c cos
    fsin = -np.sin(ang)      # Im X_f   = -sum_c q_c sin
    w = 2.0 / L              # irfft weight for interior bins
    fx = np.concatenate([fcos * w, fsin * w, fsin * w, fcos * w], axis=1)  # [64, 128]
    fy = np.concatenate([fcos, fsin, fcos, fsin], axis=1)                  # [64, 128]
    t = np.arange(L)
    angt = 2 * np.pi * np.outer(f, t) / L
    cosb, sinb = np.cos(angt), np.sin(angt)
    basis64 = np.concatenate([cosb, -sinb], axis=0)                        # [64, 2048]
    basisdup = np.concatenate([basis64, basis64], axis=0)                  # [128, 2048]
    # compression: Ccs[0:32] = P[0:32] + P[32:64]  (re*re + im*im -> cos)
    #              Ccs[32:64] = P[64:96] - P[96:128] (im*re - re*im -> -sin)
    mcomp = np.zeros((NCOMP, NCC), np.float32)
    for m in range(32):
        mcomp[m, m] = 1.0
        mcomp[m + 32, m] = 1.0
        mcomp[m + 64, m + 32] = 1.0
        mcomp[m + 96, m + 32] = -1.0
    bf = ml_dtypes.bfloat16
    return fx.astype(bf), fy.astype(bf), basisdup.astype(bf), mcomp.astype(bf)


def _build():
    exp_op = _register_exp_op()
    nc = bacc.Bacc("TRN2", target_bir_lowering=False, debug=False, num_devices=NCORES)

    qT_d = nc.dram_tensor("qT", [B, D, L], BF16, kind="ExternalInput")
    kT_d = nc.dram_tensor("kT", [B, D, L], BF16, kind="ExternalInput")
    v_d = nc.dram_tensor("v", [B, L, D], BF16, kind="ExternalInput")
    fx_d = nc.dram_tensor("fx", [D, NCOMP], BF16, kind="ExternalInput")
    fy_d = nc.dram_tensor("fy", [D, NCOMP], BF16, kind="ExternalInput")
    basis_d = nc.dram_tensor("basis2", [NCOMP, L], BF16, kind="ExternalInput")
    mcomp_d = nc.dram_tensor("mcomp", [NCOMP, NCC], BF16, kind="ExternalInput")
    out_d = nc.dram_tensor("out", [B, D, L], F32, kind="ExternalOutput")

    rg = [list(range(NCORES))]

    with tile.TileContext(nc) as tc:
        with (
            tc.tile_pool(name="consts", bufs=1) as consts,
            tc.tile_pool(name="qk", bufs=2) as qk_pool,
            tc.tile_pool(name="vv", bufs=2) as v_pool,
            tc.tile_pool(name="xy", bufs=2) as xy_pool,
            tc.tile_pool(name="cf", bufs=2) as cf_pool,
            tc.tile_pool(name="cs", bufs=2) as cs_pool,
            tc.tile_pool(name="cd", bufs=2) as cd_pool,
            tc.tile_pool(name="wts", bufs=6) as w_pool,
            tc.tile_pool(name="small", bufs=12) as s_pool,
            tc.tile_pool(name="outp", bufs=2) as out_pool,
            tc.tile_pool(name="ps_log", bufs=3, space="PSUM") as ps_log,
            tc.tile_pool(name="ps_vt", bufs=1, space="PSUM") as ps_vt,
            tc.tile_pool(name="dram", bufs=1, space="DRAM") as dram,
        ):
            fx_sb = consts.tile([D, NCOMP], BF16)
            fy_sb = consts.tile([D, NCOMP], BF16)
            basis_sb = consts.tile([NCOMP, L], BF16)
            mcomp_sb = consts.tile([NCOMP, NCC], BF16)
            nc.sync.dma_start(out=fx_sb[:], in_=fx_d[:])
            nc.sync.dma_start(out=fy_sb[:], in_=fy_d[:])
            nc.gpsimd.dma_start(out=basis_sb[:], in_=basis_d[:])
            nc.sync.dma_start(out=mcomp_sb[:], in_=mcomp_d[:])

            cc_in_h = [dram.tile([B * NCC, 1024], BF16, name=f"cc_in_h{j}")
                       for j in range(2)]
            cc_out_h = [dram.tile([B * NCC, 1024], BF16, addr_space="Shared",
                                  name=f"cc_out_h{j}") for j in range(2)]

            # Prefetch everything while the coefficient pipeline runs.
            qk_sb = []
            for b in range(B):
                qT_sb = qk_pool.tile([D, L], BF16, tag=f"qT{b}")
                kT_sb = qk_pool.tile([D, L], BF16, tag=f"kT{b}")
                nc.sync.dma_start(out=qT_sb[:], in_=qT_d[b])
                nc.sync.dma_start(out=kT_sb[:], in_=kT_d[b])
                qk_sb.append((qT_sb, kT_sb))
            v_sbs = []
            for b in range(B):
                v_sb = v_pool.tile([128, SC, D], BF16, tag=f"v{b}")
                nc.gpsimd.dma_start(
                    out=v_sb[:], in_=v_d[b].rearrange("(c p) d -> p c d", p=128)
                )
                v_sbs.append(v_sb)

            # ---- Phase 1: compressed coefficients Ccs, b-stacked [128, L] ----
            # Column-halved: the AllReduce for s-columns 0:1024 fires after the
            # first half of the pipeline, and its result is all that the first
            # 8 s-chunks of the main loop need — the second AllReduce hides
            # under main-loop compute.  b0 -> partitions 0:64, b1 -> 64:128.
            ccs_h = [cs_pool.tile([B * NCC, 1024], BF16, tag=f"ccs{j}", name=f"ccs_h{j}")
                     for j in range(2)]
            for j in range(2):  # s-column halves of 1024
                for b in range(B):
                    qT_sb, kT_sb = qk_sb[b]
                    xt2 = xy_pool.tile([NCOMP, 1024], BF16, tag="xt2")
                    yt2 = xy_pool.tile([NCOMP, 1024], BF16, tag="yt2")
                    engs = ("scalar", "vector") if b == 0 else ("vector", "scalar")
                    for src_sb, fmat, dst, cast_eng in (
                        (qT_sb, fx_sb, xt2, engs[0]),
                        (kT_sb, fy_sb, yt2, engs[1]),
                    ):
                        ps = ps_log.tile([NCOMP, 1024], F32, tag="log")
                        for q in range(2):
                            nc.tensor.matmul(
                                ps[:, q * 512:(q + 1) * 512],
                                fmat[:],
                                src_sb[:, j * 1024 + q * 512: j * 1024 + (q + 1) * 512],
                                start=True, stop=True,
                            )
                        if cast_eng == "scalar":
                            nc.scalar.copy(dst[:], ps[:])
                        else:
                            nc.vector.tensor_copy(dst[:], ps[:])

                    cf = cf_pool.tile([NCOMP, 1024], BF16, tag="cfull")
                    nc.vector.tensor_mul(cf[:], xt2[:], yt2[:])

                    ps = ps_log.tile([NCOMP, 1024], F32, tag="log")
                    pcc = ps[b * NCC:(b + 1) * NCC, :]
                    for q in range(2):
                        nc.tensor.matmul(
                            pcc[:, q * 512:(q + 1) * 512],
                            mcomp_sb[:],
                            cf[:, q * 512:(q + 1) * 512],
                            start=True, stop=True,
                        )
                    dst = ccs_h[j][b * NCC:(b + 1) * NCC, :]
                    nc.scalar.copy(dst, pcc)
                    nc.sync.dma_start(
                        out=cc_in_h[j][b * NCC:(b + 1) * NCC, :], in_=dst
                    )
                nc.gpsimd.collective_compute(
                    "AllReduce", mybir.AluOpType.add, replica_groups=rg,
                    ins=[cc_in_h[j][:].opt()], outs=[cc_out_h[j][:].opt()],
                )

            # cd = ccs - mean_h = (csum * -1/8) + ccs, duplicated to both
            # partition halves so K=64 logits matmuls row-pack the PE.
            cd2h = [[None, None], [None, None]]
            for j in range(2):
                csum = cs_pool.tile([B * NCC, 1024], BF16, tag=f"csum{j}",
                                    name=f"csum_h{j}")
                nc.sync.dma_start(out=csum[:], in_=cc_out_h[j][:])
                cda = cs_pool.tile([B * NCC, 1024], BF16, tag=f"cda{j}",
                                   name=f"cd_all{j}")
                nc.vector.scalar_tensor_tensor(
                    cda[:], csum[:], -1.0 / NCORES, ccs_h[j][:],
                    op0=mybir.AluOpType.mult, op1=mybir.AluOpType.add,
                )
                for b in range(B):
                    cdd = cd_pool.tile([2 * NCC, 1024], BF16, tag=f"cd2_{b}{j}",
                                       name=f"cd2_{b}{j}")
                    nc.sync.dma_start(out=cdd[0:NCC, :],
                                      in_=cda[b * NCC:(b + 1) * NCC, :])
                    nc.sync.dma_start(out=cdd[NCC:2 * NCC, :],
                                      in_=cda[b * NCC:(b + 1) * NCC, :])
                    cd2h[b][j] = cdd

            # ---- Phase 2: per-b softmax + delay aggregation ----
            # Vt packed: partitions 0-63 = Vt[:, 0:1024], 64-127 = Vt[:, 1024:2048]
            for b in range(B):
                v_sb = v_sbs[b]
                vt_ps = ps_vt.tile([128, 1024], F32, tag="vt")

                wts_hist = {}
                vts_hist = {}
                sig_hist = {}

                def emit_acc(sc):
                    pwt = wts_hist.pop(sc)
                    pvts = vts_hist.pop(sc)
                    for q in range(2):  # packed pairs: (q, q+2)
                        nc.tensor.matmul(
                            vt_ps[0:D, q * 512:(q + 1) * 512],
                            pvts[:],
                            pwt[0][:, q * 512:(q + 1) * 512],
                            start=(sc == 0), stop=(sc == SC - 1),
                        )
                        nc.tensor.matmul(
                            vt_ps[D:2 * D, q * 512:(q + 1) * 512],
                            pvts[:],
                            pwt[1][:, q * 512:(q + 1) * 512],
                            start=(sc == 0), stop=(sc == SC - 1),
                        )

                def emit_small(sc):
                    sig = sig_hist.pop(sc)
                    sigsum = s_pool.tile([128, 1], F32, tag="sigsum")
                    nc.gpsimd.tensor_add(sigsum[:], sig[:, 0:1], sig[:, 1:2])
                    rcp = s_pool.tile([128, 1], F32, tag="rcp")
                    nc.vector.reciprocal_approx_fast(rcp[:], sigsum[:])
                    vts = s_pool.tile([128, D], BF16, tag="vts")
                    nc.vector.tensor_scalar_mul(vts[:], v_sb[:, sc, :], rcp[:])
                    vts_hist[sc] = vts

                for sc in range(SC):
                    half = cd2h[b][sc // 8]
                    off = (sc % 8) * 128
                    cdt = half[0:NCC, off:off + 128]
                    cdb = half[NCC:2 * NCC, off:off + 128]
                    lg0 = ps_log.tile([128, 1024], F32, tag="log")
                    lg1 = ps_log.tile([128, 1024], F32, tag="log")
                    for q in range(2):
                        # row-packed pair: h2=0 on PE rows 0-63, h2=1 on 64-127
                        nc.tensor.matmul(
                            lg0[:, q * 512:(q + 1) * 512], cdt,
                            basis_sb[0:NCC, q * 512:(q + 1) * 512],
                            start=True, stop=True,
                        )
                        nc.tensor.matmul(
                            lg1[:, q * 512:(q + 1) * 512], cdb,
                            basis_sb[NCC:2 * NCC, 1024 + q * 512: 1024 + (q + 1) * 512],
                            start=True, stop=True,
                        )
                    if sc >= 2:
                        emit_acc(sc - 2)

                    sig = s_pool.tile([128, 2], F32, tag="sig")
                    wt0 = w_pool.tile([128, 1024], BF16, tag="wt")
                    nc.scalar.activation(
                        wt0[:], lg0[:], mybir.ActivationFunctionType.Exp,
                        accum_out=sig[:, 0:1],
                    )
                    wt1 = w_pool.tile([128, 1024], BF16, tag="wt")
                    nc.vector._custom_dve(
                        exp_op, out=wt1[:], in0=lg1[:],
                        s0=EXP_C[0], s1=EXP_C[1], imm2=EXP_C[2],
                        accum_out=sig[:, 1:2],
                    )
                    wts_hist[sc] = (wt0, wt1)
                    sig_hist[sc] = sig
                    if sc >= 1:
                        emit_small(sc - 1)

                emit_small(SC - 1)
                emit_acc(SC - 2)
                emit_acc(SC - 1)

                out_sb = out_pool.tile([128, 1024], F32, tag="out")
                nc.vector.tensor_copy(out_sb[:], vt_ps[:])
                nc.sync.dma_start(out=out_d[b][:, 0:1024], in_=out_sb[0:D, :])
                nc.sync.dma_start(out=out_d[b][:, 1024:2048], in_=out_sb[D:2 * D, :])

    nc.compile()
    return nc



_COMPILED_A = None
_COMPILED_B = None


def _build_split_a():
    """NEFF A: coefficient pipeline only.  Outputs b-stacked Ccs [128, L]."""
    _register_exp_op()
    nc = bacc.Bacc("TRN2", target_bir_lowering=False, debug=False, num_devices=NCORES)
    qT_d = nc.dram_tensor("qT", [B, D, L], BF16, kind="ExternalInput")
    kT_d = nc.dram_tensor("kT", [B, D, L], BF16, kind="ExternalInput")
    fx_d = nc.dram_tensor("fx", [D, NCOMP], BF16, kind="ExternalInput")
    fy_d = nc.dram_tensor("fy", [D, NCOMP], BF16, kind="ExternalInput")
    mcomp_d = nc.dram_tensor("mcomp", [NCOMP, NCC], BF16, kind="ExternalInput")
    ccs_d = nc.dram_tensor("ccs", [B * NCC, L], BF16, kind="ExternalOutput")

    with tile.TileContext(nc) as tc:
        with (
            tc.tile_pool(name="consts", bufs=1) as consts,
            tc.tile_pool(name="qk", bufs=2) as qk_pool,
            tc.tile_pool(name="xy", bufs=4) as xy_pool,
            tc.tile_pool(name="cf", bufs=4) as cf_pool,
            tc.tile_pool(name="cs", bufs=4) as cs_pool,
            tc.tile_pool(name="ps", bufs=4, space="PSUM") as ps_pool,
        ):
            fx_sb = consts.tile([D, NCOMP], BF16)
            fy_sb = consts.tile([D, NCOMP], BF16)
            mcomp_sb = consts.tile([NCOMP, NCC], BF16)
            nc.sync.dma_start(out=fx_sb[:], in_=fx_d[:])
            nc.sync.dma_start(out=fy_sb[:], in_=fy_d[:])
            nc.sync.dma_start(out=mcomp_sb[:], in_=mcomp_d[:])
            qk_sb = []
            for b in range(B):
                qT_sb = qk_pool.tile([D, L], BF16, tag=f"qT{b}")
                kT_sb = qk_pool.tile([D, L], BF16, tag=f"kT{b}")
                nc.sync.dma_start(out=qT_sb[:], in_=qT_d[b])
                nc.gpsimd.dma_start(out=kT_sb[:], in_=kT_d[b])
                qk_sb.append((qT_sb, kT_sb))

            for b in range(B):
                qT_sb, kT_sb = qk_sb[b]
                for j in range(2):
                    xt2 = xy_pool.tile([NCOMP, 1024], BF16, tag="xt2")
                    yt2 = xy_pool.tile([NCOMP, 1024], BF16, tag="yt2")
                    for src_sb, fmat, dst, cast_eng in (
                        (qT_sb, fx_sb, xt2, "scalar"),
                        (kT_sb, fy_sb, yt2, "vector"),
                    ):
                        ps = ps_pool.tile([NCOMP, 1024], F32, tag="log")
                        for q in range(2):
                            nc.tensor.matmul(
                                ps[:, q * 512:(q + 1) * 512],
                                fmat[:],
                                src_sb[:, j * 1024 + q * 512: j * 1024 + (q + 1) * 512],
                                start=True, stop=True,
                            )
                        if cast_eng == "scalar":
                            nc.scalar.copy(dst[:], ps[:])
                        else:
                            nc.vector.tensor_copy(dst[:], ps[:])
                    cf = cf_pool.tile([NCOMP, 1024], BF16, tag="cfull")
                    nc.gpsimd.tensor_mul(cf[:], xt2[:], yt2[:])
                    ps = ps_pool.tile([NCOMP, 1024], F32, tag="log")
                    pcc = ps[b * NCC:(b + 1) * NCC, :]
                    for q in range(2):
                        nc.tensor.matmul(
                            pcc[:, q * 512:(q + 1) * 512],
                            mcomp_sb[:],
                            cf[:, q * 512:(q + 1) * 512],
                            start=True, stop=True,
                        )
                    ccs = cs_pool.tile([NCC, 1024], BF16, tag="ccs")
                    if (b + j) % 2 == 0:
                        nc.vector.tensor_copy(ccs[:], pcc)
                    else:
                        nc.scalar.copy(ccs[:], pcc)
                    nc.sync.dma_start(
                        out=ccs_d[b * NCC:(b + 1) * NCC, j * 1024:(j + 1) * 1024],
                        in_=ccs[:],
                    )
    nc.compile()
    return nc


def _build_split_b():
    """NEFF B: softmax + delay aggregation from host-reduced coefficients."""
    exp_op = _register_exp_op()
    nc = bacc.Bacc("TRN2", target_bir_lowering=False, debug=False, num_devices=NCORES)
    cd_d = nc.dram_tensor("cd2", [B, 2 * NCC, L], BF16, kind="ExternalInput")
    v_d = nc.dram_tensor("v", [B, L, D], BF16, kind="ExternalInput")
    basis_d = nc.dram_tensor("basis2", [NCOMP, L], BF16, kind="ExternalInput")
    out_d = nc.dram_tensor("out", [B, D, L], F32, kind="ExternalOutput")

    with tile.TileContext(nc) as tc:
        with (
            tc.tile_pool(name="consts", bufs=1) as consts,
            tc.tile_pool(name="vv", bufs=2) as v_pool,
            tc.tile_pool(name="cd", bufs=2) as cd_pool,
            tc.tile_pool(name="wts", bufs=10) as w_pool,
            tc.tile_pool(name="small", bufs=12) as s_pool,
            tc.tile_pool(name="outp", bufs=2) as out_pool,
            tc.tile_pool(name="ps_log", bufs=3, space="PSUM") as ps_log,
            tc.tile_pool(name="ps_vt", bufs=1, space="PSUM") as ps_vt,
        ):
            basis_sb = consts.tile([NCOMP, L], BF16)
            nc.sync.dma_start(out=basis_sb[:], in_=basis_d[:])
            cd_sbs = []
            v_sbs = []
            for b in range(B):
                quarters = []
                for j in range(4):
                    cdd = cd_pool.tile([2 * NCC, 512], BF16, tag=f"cd{b}{j}")
                    nc.gpsimd.dma_start(out=cdd[:],
                                        in_=cd_d[b][:, j * 512:(j + 1) * 512])
                    quarters.append(cdd)
                cd_sbs.append(quarters)
                v_sb = v_pool.tile([128, SC, D], BF16, tag=f"v{b}")
                nc.sync.dma_start(
                    out=v_sb[:], in_=v_d[b].rearrange("(c p) d -> p c d", p=128)
                )
                v_sbs.append(v_sb)

            for b in range(B):
                v_sb = v_sbs[b]
                vt_ps = ps_vt.tile([128, 1024], F32, tag="vt")
                wts_hist = {}
                vts_hist = {}
                sig_hist = {}

                def emit_acc(sc):
                    pwt = wts_hist.pop(sc)
                    pvts = vts_hist.pop(sc)
                    for q in range(2):
                        nc.tensor.matmul(
                            vt_ps[0:D, q * 512:(q + 1) * 512],
                            pvts[:],
                            pwt[0][:, q * 512:(q + 1) * 512],
                            start=(sc == 0), stop=(sc == SC - 1),
                        )
                        nc.tensor.matmul(
                            vt_ps[D:2 * D, q * 512:(q + 1) * 512],
                            pvts[:],
                            pwt[1][:, q * 512:(q + 1) * 512],
                            start=(sc == 0), stop=(sc == SC - 1),
                        )

                def emit_small(sc):
                    sig = sig_hist.pop(sc)
                    sigsum = s_pool.tile([128, 1], F32, tag="sigsum")
                    nc.gpsimd.tensor_add(sigsum[:], sig[:, 0:1], sig[:, 1:2])
                    rcp = s_pool.tile([128, 1], F32, tag="rcp")
                    nc.vector.reciprocal_approx_fast(rcp[:], sigsum[:])
                    vts = s_pool.tile([128, D], BF16, tag="vts")
                    nc.gpsimd.tensor_scalar_mul(vts[:], v_sb[:, sc, :], rcp[:])
                    vts_hist[sc] = vts

                for sc in range(SC):
                    quarter = cd_sbs[b][sc // 4]
                    off = (sc % 4) * 128
                    cdt = quarter[0:NCC, off:off + 128]
                    cdb = quarter[NCC:2 * NCC, off:off + 128]
                    lg0 = ps_log.tile([128, 1024], F32, tag="log")
                    lg1 = ps_log.tile([128, 1024], F32, tag="log")
                    for q in range(2):
                        nc.tensor.matmul(
                            lg0[:, q * 512:(q + 1) * 512], cdt,
                            basis_sb[0:NCC, q * 512:(q + 1) * 512],
                            start=True, stop=True,
                        )
                        nc.tensor.matmul(
                            lg1[:, q * 512:(q + 1) * 512], cdb,
                            basis_sb[NCC:2 * NCC, 1024 + q * 512: 1024 + (q + 1) * 512],
                            start=True, stop=True,
                        )
                    if sc >= 2:
                        emit_acc(sc - 2)
                    sig = s_pool.tile([128, 2], F32, tag="sig")
                    wt0 = w_pool.tile([128, 1024], BF16, tag="wt")
                    nc.scalar.activation(
                        wt0[:], lg0[:], mybir.ActivationFunctionType.Exp,
                        accum_out=sig[:, 0:1],
                    )
                    wt1 = w_pool.tile([128, 1024], BF16, tag="wt")
                    nc.vector._custom_dve(
                        exp_op, out=wt1[:], in0=lg1[:],
                        s0=EXP_C[0], s1=EXP_C[1], imm2=EXP_C[2],
                        accum_out=sig[:, 1:2],
                    )
                    wts_hist[sc] = (wt0, wt1)
                    sig_hist[sc] = sig
                    if sc >= 1:
                        emit_small(sc - 1)

                emit_small(SC - 1)
                emit_acc(SC - 2)
                emit_acc(SC - 1)

                out_sb = out_pool.tile([128, 1024], F32, tag="out")
                nc.vector.tensor_copy(out_sb[:], vt_ps[:])
                nc.sync.dma_start(out=out_d[b][:, 0:1024], in_=out_sb[0:D, :])
                nc.sync.dma_start(out=out_d[b][:, 1024:2048], in_=out_sb[D:2 * D, :])
    nc.compile()
    return nc


def _get_split():
    global _COMPILED_A, _COMPILED_B
    if _COMPILED_A is None:
        _COMPILED_A = _build_split_a()
        _COMPILED_B = _build_split_b()
    return _COMPILED_A, _COMPILED_B


def _get_compiled():
    global _COMPILED
    if _COMPILED is None:
        _COMPILED = _build()
    return _COMPILED


def kernel(queries, keys, values):
    global LAST_RESULT
    queries = np.asarray(queries, dtype=np.float32)
    keys = np.asarray(keys, dtype=np.float32)
    values = np.asarray(values, dtype=np.float32)

    fx, fy, basisdup, mcomp = _constants()
    bf = ml_dtypes.bfloat16

    in_maps = []
    for i in range(NCORES):
        sl = slice(i * D, (i + 1) * D)
        in_maps.append({
            "qT": np.ascontiguousarray(queries[:, :, sl].transpose(0, 2, 1)).astype(bf),
            "kT": np.ascontiguousarray(keys[:, :, sl].transpose(0, 2, 1)).astype(bf),
            "v": np.ascontiguousarray(values[:, :, sl]).astype(bf),
            "fx": fx,
            "fy": fy,
            "basis2": basisdup,
            "mcomp": mcomp,
        })

    kw = {"trace_cores": list(range(NCORES))} if TRACE else {}
    cores = list(range(NCORES))
    if SPLIT:
        nca, ncb = _get_split()
        maps_a = [{k: m[k] for k in ("qT", "kT", "fx", "fy", "mcomp")}
                  for m in in_maps]
        res_a = run_bass_kernel_spmd(nca, maps_a, core_ids=cores, trace=TRACE, **kw)
        ccs_all = np.stack([res_a.results[i]["ccs"] for i in range(NCORES)])
        csum = ccs_all.astype(np.float32).sum(axis=0) * (1.0 / NCORES)
        maps_b = []
        for i in range(NCORES):
            cd_all = (ccs_all[i].astype(np.float32) - csum).astype(bf)  # [128, L]
            cd2 = np.stack([np.concatenate([cd_all[b * NCC:(b + 1) * NCC]] * 2, axis=0)
                            for b in range(B)])                          # [B, 128, L]
            maps_b.append({"cd2": cd2, "v": in_maps[i]["v"],
                           "basis2": in_maps[i]["basis2"]})
        res = run_bass_kernel_spmd(ncb, maps_b, core_ids=cores, trace=TRACE, **kw)
        LAST_RESULT = res
        globals()["LAST_RESULT_A"] = res_a
    else:
        nc = _get_compiled()
        res = run_bass_kernel_spmd(nc, in_maps, core_ids=cores, trace=TRACE, **kw)
        LAST_RESULT = res

    vt_full = np.stack([res.results[i]["out"] for i in range(NCORES)], axis=1)
    # reference: out = transpose(Vt[B,H,d,L], (0,2,1,3)).reshape(B, L, H*d)
    return np.ascontiguousarray(
        vt_full.transpose(0, 2, 1, 3).reshape(B, L, E)
    ).astype(np.float32)


# revision 26
# speedup vs baseline: 1.2530x; 1.2530x over previous
"""AutoCorrelation (Autoformer-style) Bass kernel for one TRN2 chip (8 NeuronCores).

Math: the reference computes, per (b, h):
    corr = irfft(rfft(q, axis=-1) * conj(rfft(k, axis=-1)), n=L)   # [L, L]
    weights = softmax(corr - mean_h(corr), axis=-1)
    Vt = v @ weights                                                # [d, L]
The rfft runs over the d=64 channel axis and the irfft zero-pads 33 bins to
L=2048, so corr[s, :] is a rank-<=66 function of t; the DC term is constant
over t and cancels in softmax.  Collapsing the spectral products
(re*re + im*im -> cos row, im*re - re*im -> sin row) leaves 64 coefficient
rows: the logits are an exact K=64 matmul against a fixed cos/sin basis and
no [L, L] tensor ever exists in DRAM.

Sharding: head h -> core h (both batches per core).  Only the head-mean of
the 64 x 2048 coefficient matrix couples cores.  Default mode (SPLIT=True)
runs two NEFFs: phase A computes coefficients (~40 us), the host sums the
8 cores' 0.5 MB outputs, and phase B (~90 us) does softmax + aggregation —
this is much faster than an on-device AllReduce, which costs 55-60 us of
mostly-fixed latency on this platform (SPLIT=False keeps everything on
device in one NEFF with column-halved AllReduces, ~170 us).

Phase B details: K=64 logits matmuls are row-packed (two concurrent 64-row
PE tiles via base_partition 0/64 of duplicated coefficient/basis tensors);
the delay-aggregation matmuls are column-packed (Vt stored [128, 1024]:
partitions 0-63 hold t 0:1024, partitions 64-127 hold t 1024:2048).  The
softmax exp splits between ScalarE (table exp) and VectorE (custom DVE op
EXP8_ANT: exp(x) ~= (c0 + x(c1 + x c2))^8, valid since logits are bounded
by ~1.5), both with fused free-dim accumulation for the denominator; the
per-row 1/sum folds into the tiny v-tile instead of the weight tile.
"""
import sys
from operator import add as _op_add

sys.path.insert(0, "/opt/trn_rl_repo")

import numpy as np
import ml_dtypes

from concourse import bass, bacc, mybir, tile
from concourse import dve_ops
from concourse.dve_spec import Spec, Src0, C0, C1, C2, Zero, sq, lower
from concourse.dve_uop import DveOpSpec
from concourse.bass_utils import run_bass_kernel_spmd

B, L, E, H, D = 2, 2048, 512, 8, 64
NF = 32          # frequencies 1..32 of the 64-point rfft (DC dropped)
NCOMP = 4 * NF   # 128 raw product rows
NCC = 2 * NF     # 64 compressed coefficient rows (cos, sin)
NCORES = 8
SC = L // 128    # 16 s-chunks of 128 rows
BF16 = mybir.dt.bfloat16
F32 = mybir.dt.float32

# minimax quadratic p(z) for e^z on z = x/8, |x| <= 1.68; exp(x) ~= p(x)^8
EXP_C = (0.99970171, 0.12580122, 0.00795605)

TRACE = False
SPLIT = True
LAST_RESULT = None
LAST_RESULT_A = None

_COMPILED = None
_EXP_OP = None


def _register_exp_op():
    global _EXP_OP
    if _EXP_OP is not None:
        return _EXP_OP
    for o in dve_ops.OPS:
        if o.name == "EXP8_ANT":
            _EXP_OP = o
            return o

    body = sq(sq(sq(C0 + Src0 * (C1 + Src0 * C2))))

    def _ref(in0, in1, c0, c1, c2):
        x = in0.astype(np.float32)
        b = (((c0 + x * (c1 + x * c2)) ** 8)).astype(np.float32)
        return b, b.reshape(b.shape[0], -1).sum(axis=-1, keepdims=True)

    spec = Spec(body=body, accum=_op_add, accum_init=Zero, reference=_ref)
    opcode = dve_ops._CUSTOM_DVE_ROW_BASE + len(dve_ops.OPS)
    dve_ops._SUB_OPCODE_FOR_NAME["EXP8_ANT"] = opcode
    shas = {}
    for ver in ("v3", "v4"):
        shas[ver] = DveOpSpec(
            name="EXP8_ANT", opcode=opcode, uops=lower(spec, ver=ver), rd1_en=False
        ).sha(ver)
    op = dve_ops.DveOp("EXP8_ANT", spec, subdim=False, uops_sha=shas)
    dve_ops.OPS.append(op)
    dve_ops.CUSTOM_DVE_SPECS[op.name] = spec
    _EXP_OP = op
    return op


def _constants():
    c = np.arange(D)
    f = np.arange(1, NF + 1)
    ang = 2 * np.pi * np.outer(c, f) / D
    fcos = np.cos(ang)       # Re X_f   = sum_c q_c cos
    fsin = -np.sin(ang)      # Im X_f   = -sum_c q_c sin
    w = 2.0 / L              # irfft weight for interior bins
    fx = np.concatenate([fcos * w, fsin * w, fsin * w, fcos * w], axis=1)  # [64, 128]
    fy = np.concatenate([fcos, fsin, fcos, fsin], axis=1)                  # [64, 128]
    t = np.arange(L)
    angt = 2 * np.pi * np.outer(f, t) / L
    cosb, sinb = np.cos(angt), np.sin(angt)
    basis64 = np.concatenate([cosb, -sinb], axis=0)                        # [64, 2048]
    basisdup = np.concatenate([basis64, basis64], axis=0)                  # [128, 2048]
    # compression: Ccs[0:32] = P[0:32] + P[32:64]  (re*re + im*im -> cos)
    #              Ccs[32:64] = P[64:96] - P[96:128] (im*re - re*im -> -sin)
    mcomp = np.zeros((NCOMP, NCC), np.float32)
    for m in range(32):
        mcomp[m, m] = 1.0
        mcomp[m + 32, m] = 1.0
        mcomp[m + 64, m + 32] = 1.0
        mcomp[m + 96, m + 32] = -1.0
    bf = ml_dtypes.bfloat16
    return fx.astype(bf), fy.astype(bf), basisdup.astype(bf), mcomp.astype(bf)


def _build():
    exp_op = _register_exp_op()
    nc = bacc.Bacc("TRN2", target_bir_lowering=False, debug=False, num_devices=NCORES)

    qT_d = nc.dram_tensor("qT", [B, D, L], BF16, kind="ExternalInput")
    kT_d = nc.dram_tensor("kT", [B, D, L], BF16, kind="ExternalInput")
    v_d = nc.dram_tensor("v", [B, L, D], BF16, kind="ExternalInput")
    fx_d = nc.dram_tensor("fx", [D, NCOMP], BF16, kind="ExternalInput")
    fy_d = nc.dram_tensor("fy", [D, NCOMP], BF16, kind="ExternalInput")
    basis_d = nc.dram_tensor("basis2", [NCOMP, L], BF16, kind="ExternalInput")
    mcomp_d = nc.dram_tensor("mcomp", [NCOMP, NCC], BF16, kind="ExternalInput")
    out_d = nc.dram_tensor("out", [B, D, L], F32, kind="ExternalOutput")

    rg = [list(range(NCORES))]

    with tile.TileContext(nc) as tc:
        with (
            tc.tile_pool(name="consts", bufs=1) as consts,
            tc.tile_pool(name="qk", bufs=2) as qk_pool,
            tc.tile_pool(name="vv", bufs=2) as v_pool,
            tc.tile_pool(name="xy", bufs=2) as xy_pool,
            tc.tile_pool(name="cf", bufs=2) as cf_pool,
            tc.tile_pool(name="cs", bufs=2) as cs_pool,
            tc.tile_pool(name="cd", bufs=2) as cd_pool,
            tc.tile_pool(name="wts", bufs=6) as w_pool,
            tc.tile_pool(name="small", bufs=12) as s_pool,
            tc.tile_pool(name="outp", bufs=2) as out_pool,
            tc.tile_pool(name="ps_log", bufs=3, space="PSUM") as ps_log,
            tc.tile_pool(name="ps_vt", bufs=1, space="PSUM") as ps_vt,
            tc.tile_pool(name="dram", bufs=1, space="DRAM") as dram,
        ):
            fx_sb = consts.tile([D, NCOMP], BF16)
            fy_sb = consts.tile([D, NCOMP], BF16)
            basis_sb = consts.tile([NCOMP, L], BF16)
            mcomp_sb = consts.tile([NCOMP, NCC], BF16)
            nc.sync.dma_start(out=fx_sb[:], in_=fx_d[:])
            nc.sync.dma_start(out=fy_sb[:], in_=fy_d[:])
            nc.gpsimd.dma_start(out=basis_sb[:], in_=basis_d[:])
            nc.sync.dma_start(out=mcomp_sb[:], in_=mcomp_d[:])

            cc_in_h = [dram.tile([B * NCC, 1024], BF16, name=f"cc_in_h{j}")
                       for j in range(2)]
            cc_out_h = [dram.tile([B * NCC, 1024], BF16, addr_space="Shared",
                                  name=f"cc_out_h{j}") for j in range(2)]

            # Prefetch everything while the coefficient pipeline runs.
            qk_sb = []
            for b in range(B):
                qT_sb = qk_pool.tile([D, L], BF16, tag=f"qT{b}")
                kT_sb = qk_pool.tile([D, L], BF16, tag=f"kT{b}")
                nc.sync.dma_start(out=qT_sb[:], in_=qT_d[b])
                nc.sync.dma_start(out=kT_sb[:], in_=kT_d[b])
                qk_sb.append((qT_sb, kT_sb))
            v_sbs = []
            for b in range(B):
                v_sb = v_pool.tile([128, SC, D], BF16, tag=f"v{b}")
                nc.gpsimd.dma_start(
                    out=v_sb[:], in_=v_d[b].rearrange("(c p) d -> p c d", p=128)
                )
                v_sbs.append(v_sb)

            # ---- Phase 1: compressed coefficients Ccs, b-stacked [128, L] ----
            # Column-halved: the AllReduce for s-columns 0:1024 fires after the
            # first half of the pipeline, and its result is all that the first
            # 8 s-chunks of the main loop need — the second AllReduce hides
            # under main-loop compute.  b0 -> partitions 0:64, b1 -> 64:128.
            ccs_h = [cs_pool.tile([B * NCC, 1024], BF16, tag=f"ccs{j}", name=f"ccs_h{j}")
                     for j in range(2)]
            for j in range(2):  # s-column halves of 1024
                for b in range(B):
                    qT_sb, kT_sb = qk_sb[b]
                    xt2 = xy_pool.tile([NCOMP, 1024], BF16, tag="xt2")
                    yt2 = xy_pool.tile([NCOMP, 1024], BF16, tag="yt2")
                    engs = ("scalar", "vector") if b == 0 else ("vector", "scalar")
                    for src_sb, fmat, dst, cast_eng in (
                        (qT_sb, fx_sb, xt2, engs[0]),
                        (kT_sb, fy_sb, yt2, engs[1]),
                    ):
                        ps = ps_log.tile([NCOMP, 1024], F32, tag="log")
                        for q in range(2):
                            nc.tensor.matmul(
                                ps[:, q * 512:(q + 1) * 512],
                                fmat[:],
                                src_sb[:, j * 1024 + q * 512: j * 1024 + (q + 1) * 512],
                                start=True, stop=True,
                            )
                        if cast_eng == "scalar":
                            nc.scalar.copy(dst[:], ps[:])
                        else:
                            nc.vector.tensor_copy(dst[:], ps[:])

                    cf = cf_pool.tile([NCOMP, 1024], BF16, tag="cfull")
                    nc.vector.tensor_mul(cf[:], xt2[:], yt2[:])

                    ps = ps_log.tile([NCOMP, 1024], F32, tag="log")
                    pcc = ps[b * NCC:(b + 1) * NCC, :]
                    for q in range(2):
                        nc.tensor.matmul(
                            pcc[:, q * 512:(q + 1) * 512],
                            mcomp_sb[:],
                            cf[:, q * 512:(q + 1) * 512],
                            start=True, stop=True,
                        )
                    dst = ccs_h[j][b * NCC:(b + 1) * NCC, :]
                    nc.scalar.copy(dst, pcc)
                    nc.sync.dma_start(
                        out=cc_in_h[j][b * NCC:(b + 1) * NCC, :], in_=dst
                    )
                nc.gpsimd.collective_compute(
                    "AllReduce", mybir.AluOpType.add, replica_groups=rg,
                    ins=[cc_in_h[j][:].opt()], outs=[cc_out_h[j][:].opt()],
                )

            # cd = ccs - mean_h = (csum * -1/8) + ccs, duplicated to both
            # partition halves so K=64 logits matmuls row-pack the PE.
            cd2h = [[None, None], [None, None]]
            for j in range(2):
                csum = cs_pool.tile([B * NCC, 1024], BF16, tag=f"csum{j}",
                                    name=f"csum_h{j}")
                nc.sync.dma_start(out=csum[:], in_=cc_out_h[j][:])
                cda = cs_pool.tile([B * NCC, 1024], BF16, tag=f"cda{j}",
                                   name=f"cd_all{j}")
                nc.vector.scalar_tensor_tensor(
                    cda[:], csum[:], -1.0 / NCORES, ccs_h[j][:],
                    op0=mybir.AluOpType.mult, op1=mybir.AluOpType.add,
                )
                for b in range(B):
                    cdd = cd_pool.tile([2 * NCC, 1024], BF16, tag=f"cd2_{b}{j}",
                                       name=f"cd2_{b}{j}")
                    nc.sync.dma_start(out=cdd[0:NCC, :],
                                      in_=cda[b * NCC:(b + 1) * NCC, :])
                    nc.sync.dma_start(out=cdd[NCC:2 * NCC, :],
                                      in_=cda[b * NCC:(b + 1) * NCC, :])
                    cd2h[b][j] = cdd

            # ---- Phase 2: per-b softmax + delay aggregation ----
            # Vt packed: partitions 0-63 = Vt[:, 0:1024], 64-127 = Vt[:, 1024:2048]
            for b in range(B):
                v_sb = v_sbs[b]
                vt_ps = ps_vt.tile([128, 1024], F32, tag="vt")

                wts_hist = {}
                vts_hist = {}
                sig_hist = {}

                def emit_acc(sc):
                    pwt = wts_hist.pop(sc)
                    pvts = vts_hist.pop(sc)
                    for q in range(2):  # packed pairs: (q, q+2)
                        nc.tensor.matmul(
                            vt_ps[0:D, q * 512:(q + 1) * 512],
                            pvts[:],
                            pwt[0][:, q * 512:(q + 1) * 512],
                            start=(sc == 0), stop=(sc == SC - 1),
                        )
                        nc.tensor.matmul(
                            vt_ps[D:2 * D, q * 512:(q + 1) * 512],
                            pvts[:],
                            pwt[1][:, q * 512:(q + 1) * 512],
                            start=(sc == 0), stop=(sc == SC - 1),
                        )

                def emit_small(sc):
                    sig = sig_hist.pop(sc)
                    sigsum = s_pool.tile([128, 1], F32, tag="sigsum")
                    nc.gpsimd.tensor_add(sigsum[:], sig[:, 0:1], sig[:, 1:2])
                    rcp = s_pool.tile([128, 1], F32, tag="rcp")
                    nc.vector.reciprocal_approx_fast(rcp[:], sigsum[:])
                    vts = s_pool.tile([128, D], BF16, tag="vts")
                    nc.vector.tensor_scalar_mul(vts[:], v_sb[:, sc, :], rcp[:])
                    vts_hist[sc] = vts

                for sc in range(SC):
                    half = cd2h[b][sc // 8]
                    off = (sc % 8) * 128
                    cdt = half[0:NCC, off:off + 128]
                    cdb = half[NCC:2 * NCC, off:off + 128]
                    lg0 = ps_log.tile([128, 1024], F32, tag="log")
                    lg1 = ps_log.tile([128, 1024], F32, tag="log")
                    for q in range(2):
                        # row-packed pair: h2=0 on PE rows 0-63, h2=1 on 64-127
                        nc.tensor.matmul(
                            lg0[:, q * 512:(q + 1) * 512], cdt,
                            basis_sb[0:NCC, q * 512:(q + 1) * 512],
                            start=True, stop=True,
                        )
                        nc.tensor.matmul(
                            lg1[:, q * 512:(q + 1) * 512], cdb,
                            basis_sb[NCC:2 * NCC, 1024 + q * 512: 1024 + (q + 1) * 512],
                            start=True, stop=True,
                        )
                    if sc >= 2:
                        emit_acc(sc - 2)

                    sig = s_pool.tile([128, 2], F32, tag="sig")
                    wt0 = w_pool.tile([128, 1024], BF16, tag="wt")
                    nc.scalar.activation(
                        wt0[:], lg0[:], mybir.ActivationFunctionType.Exp,
                        accum_out=sig[:, 0:1],
                    )
                    wt1 = w_pool.tile([128, 1024], BF16, tag="wt")
                    nc.vector._custom_dve(
                        exp_op, out=wt1[:], in0=lg1[:],
                        s0=EXP_C[0], s1=EXP_C[1], imm2=EXP_C[2],
                        accum_out=sig[:, 1:2],
                    )
                    wts_hist[sc] = (wt0, wt1)
                    sig_hist[sc] = sig
                    if sc >= 1:
                        emit_small(sc - 1)

                emit_small(SC - 1)
                emit_acc(SC - 2)
                emit_acc(SC - 1)

                out_sb = out_pool.tile([128, 1024], F32, tag="out")
                nc.vector.tensor_copy(out_sb[:], vt_ps[:])
                nc.sync.dma_start(out=out_d[b][:, 0:1024], in_=out_sb[0:D, :])
                nc.sync.dma_start(out=out_d[b][:, 1024:2048], in_=out_sb[D:2 * D, :])

    nc.compile()
    return nc



_COMPILED_A = None
_COMPILED_B = None


def _build_split_a():
    """NEFF A: coefficient pipeline only.  Outputs b-stacked Ccs [128, L]."""
    _register_exp_op()
    nc = bacc.Bacc("TRN2", target_bir_lowering=False, debug=False, num_devices=NCORES)
    qT_d = nc.dram_tensor("qT", [B, D, L], BF16, kind="ExternalInput")
    kT_d = nc.dram_tensor("kT", [B, D, L], BF16, kind="ExternalInput")
    fx_d = nc.dram_tensor("fx", [D, NCOMP], BF16, kind="ExternalInput")
    fy_d = nc.dram_tensor("fy", [D, NCOMP], BF16, kind="ExternalInput")
    mcomp_d = nc.dram_tensor("mcomp", [NCOMP, NCC], BF16, kind="ExternalInput")
    ccs_d = nc.dram_tensor("ccs", [B * NCC, L], BF16, kind="ExternalOutput")

    with tile.TileContext(nc) as tc:
        with (
            tc.tile_pool(name="consts", bufs=1) as consts,
            tc.tile_pool(name="qk", bufs=2) as qk_pool,
            tc.tile_pool(name="xy", bufs=4) as xy_pool,
            tc.tile_pool(name="cf", bufs=4) as cf_pool,
            tc.tile_pool(name="cs", bufs=4) as cs_pool,
            tc.tile_pool(name="ps", bufs=4, space="PSUM") as ps_pool,
        ):
            fx_sb = consts.tile([D, NCOMP], BF16)
            fy_sb = consts.tile([D, NCOMP], BF16)
            mcomp_sb = consts.tile([NCOMP, NCC], BF16)
            nc.sync.dma_start(out=fx_sb[:], in_=fx_d[:])
            nc.sync.dma_start(out=fy_sb[:], in_=fy_d[:])
            nc.sync.dma_start(out=mcomp_sb[:], in_=mcomp_d[:])
            qk_sb = []
            for b in range(B):
                qT_sb = qk_pool.tile([D, L], BF16, tag=f"qT{b}")
                kT_sb = qk_pool.tile([D, L], BF16, tag=f"kT{b}")
                nc.sync.dma_start(out=qT_sb[:], in_=qT_d[b])
                nc.sync.dma_start(out=kT_sb[:], in_=kT_d[b])
                qk_sb.append((qT_sb, kT_sb))

            for b in range(B):
                qT_sb, kT_sb = qk_sb[b]
                for j in range(2):
                    xt2 = xy_pool.tile([NCOMP, 1024], BF16, tag="xt2")
                    yt2 = xy_pool.tile([NCOMP, 1024], BF16, tag="yt2")
                    engs = ("scalar", "vector") if b == 0 else ("vector", "scalar")
                    for src_sb, fmat, dst, cast_eng in (
                        (qT_sb, fx_sb, xt2, engs[0]),
                        (kT_sb, fy_sb, yt2, engs[1]),
                    ):
                        ps = ps_pool.tile([NCOMP, 1024], F32, tag="log")
                        for q in range(2):
                            nc.tensor.matmul(
                                ps[:, q * 512:(q + 1) * 512],
                                fmat[:],
                                src_sb[:, j * 1024 + q * 512: j * 1024 + (q + 1) * 512],
                                start=True, stop=True,
                            )
                        if cast_eng == "scalar":
                            nc.scalar.copy(dst[:], ps[:])
                        else:
                            nc.vector.tensor_copy(dst[:], ps[:])
                    cf = cf_pool.tile([NCOMP, 1024], BF16, tag="cfull")
                    nc.vector.tensor_mul(cf[:], xt2[:], yt2[:])
                    ps = ps_pool.tile([NCOMP, 1024], F32, tag="log")
                    pcc = ps[b * NCC:(b + 1) * NCC, :]
                    for q in range(2):
                        nc.tensor.matmul(
                            pcc[:, q * 512:(q + 1) * 512],
                            mcomp_sb[:],
                            cf[:, q * 512:(q + 1) * 512],
                            start=True, stop=True,
                        )
                    ccs = cs_pool.tile([NCC, 1024], BF16, tag="ccs")
                    nc.scalar.copy(ccs[:], pcc)
                    nc.sync.dma_start(
                        out=ccs_d[b * NCC:(b + 1) * NCC, j * 1024:(j + 1) * 1024],
                        in_=ccs[:],
                    )
    nc.compile()
    return nc


def _build_split_b():
    """NEFF B: softmax + delay aggregation from host-reduced coefficients."""
    exp_op = _register_exp_op()
    nc = bacc.Bacc("TRN2", target_bir_lowering=False, debug=False, num_devices=NCORES)
    cd_d = nc.dram_tensor("cd2", [B, 2 * NCC, L], BF16, kind="ExternalInput")
    v_d = nc.dram_tensor("v", [B, L, D], BF16, kind="ExternalInput")
    basis_d = nc.dram_tensor("basis2", [NCOMP, L], BF16, kind="ExternalInput")
    out_d = nc.dram_tensor("out", [B, D, L], F32, kind="ExternalOutput")

    with tile.TileContext(nc) as tc:
        with (
            tc.tile_pool(name="consts", bufs=1) as consts,
            tc.tile_pool(name="vv", bufs=2) as v_pool,
            tc.tile_pool(name="cd", bufs=2) as cd_pool,
            tc.tile_pool(name="wts", bufs=10) as w_pool,
            tc.tile_pool(name="small", bufs=12) as s_pool,
            tc.tile_pool(name="outp", bufs=2) as out_pool,
            tc.tile_pool(name="ps_log", bufs=3, space="PSUM") as ps_log,
            tc.tile_pool(name="ps_vt", bufs=1, space="PSUM") as ps_vt,
        ):
            basis_sb = consts.tile([NCOMP, L], BF16)
            nc.sync.dma_start(out=basis_sb[:], in_=basis_d[:])
            cd_sbs = []
            v_sbs = []
            for b in range(B):
                halves = []
                for j in range(2):
                    cdd = cd_pool.tile([2 * NCC, 1024], BF16, tag=f"cd{b}{j}")
                    nc.sync.dma_start(out=cdd[:], in_=cd_d[b][:, j * 1024:(j + 1) * 1024])
                    halves.append(cdd)
                cd_sbs.append(halves)
                v_sb = v_pool.tile([128, SC, D], BF16, tag=f"v{b}")
                nc.gpsimd.dma_start(
                    out=v_sb[:], in_=v_d[b].rearrange("(c p) d -> p c d", p=128)
                )
                v_sbs.append(v_sb)

            for b in range(B):
                v_sb = v_sbs[b]
                vt_ps = ps_vt.tile([128, 1024], F32, tag="vt")
                wts_hist = {}
                vts_hist = {}
                sig_hist = {}

                def emit_acc(sc):
                    pwt = wts_hist.pop(sc)
                    pvts = vts_hist.pop(sc)
                    for q in range(2):
                        nc.tensor.matmul(
                            vt_ps[0:D, q * 512:(q + 1) * 512],
                            pvts[:],
                            pwt[0][:, q * 512:(q + 1) * 512],
                            start=(sc == 0), stop=(sc == SC - 1),
                        )
                        nc.tensor.matmul(
                            vt_ps[D:2 * D, q * 512:(q + 1) * 512],
                            pvts[:],
                            pwt[1][:, q * 512:(q + 1) * 512],
                            start=(sc == 0), stop=(sc == SC - 1),
                        )

                def emit_small(sc):
                    sig = sig_hist.pop(sc)
                    sigsum = s_pool.tile([128, 1], F32, tag="sigsum")
                    nc.gpsimd.tensor_add(sigsum[:], sig[:, 0:1], sig[:, 1:2])
                    rcp = s_pool.tile([128, 1], F32, tag="rcp")
                    nc.vector.reciprocal_approx_fast(rcp[:], sigsum[:])
                    vts = s_pool.tile([128, D], BF16, tag="vts")
                    nc.gpsimd.tensor_scalar_mul(vts[:], v_sb[:, sc, :], rcp[:])
                    vts_hist[sc] = vts

                for sc in range(SC):
                    half = cd_sbs[b][sc // 8]
                    off = (sc % 8) * 128
                    cdt = half[0:NCC, off:off + 128]
                    cdb = half[NCC:2 * NCC, off:off + 128]
                    lg0 = ps_log.tile([128, 1024], F32, tag="log")
                    lg1 = ps_log.tile([128, 1024], F32, tag="log")
                    for q in range(2):
                        nc.tensor.matmul(
                            lg0[:, q * 512:(q + 1) * 512], cdt,
                            basis_sb[0:NCC, q * 512:(q + 1) * 512],
                            start=True, stop=True,
                        )
                        nc.tensor.matmul(
                            lg1[:, q * 512:(q + 1) * 512], cdb,
                            basis_sb[NCC:2 * NCC, 1024 + q * 512: 1024 + (q + 1) * 512],
                            start=True, stop=True,
                        )
                    if sc >= 2:
                        emit_acc(sc - 2)
                    sig = s_pool.tile([128, 2], F32, tag="sig")
                    wt0 = w_pool.tile([128, 1024], BF16, tag="wt")
                    nc.scalar.activation(
                        wt0[:], lg0[:], mybir.ActivationFunctionType.Exp,
                        accum_out=sig[:, 0:1],
                    )
                    wt1 = w_pool.tile([128, 1024], BF16, tag="wt")
                    nc.vector._custom_dve(
                        exp_op, out=wt1[:], in0=lg1[:],
                        s0=EXP_C[0], s1=EXP_C[1], imm2=EXP_C[2],
                        accum_out=sig[:, 1:2],
                    )
                    wts_hist[sc] = (wt0, wt1)
                    sig_hist[sc] = sig
                    if sc >= 1:
                        emit_small(sc - 1)

                emit_small(SC - 1)
                emit_acc(SC - 2)
                emit_acc(SC - 1)

                out_sb = out_pool.tile([128, 1024], F32, tag="out")
                nc.vector.tensor_copy(out_sb[:], vt_ps[:])
                nc.sync.dma_start(out=out_d[b][:, 0:1024], in_=out_sb[0:D, :])
                nc.sync.dma_start(out=out_d[b][:, 1024:2048], in_=out_sb[D:2 * D, :])
    nc.compile()
    return nc


def _get_split():
    global _COMPILED_A, _COMPILED_B
    if _COMPILED_A is None:
        _COMPILED_A = _build_split_a()
        _COMPILED_B = _build_split_b()
    return _COMPILED_A, _COMPILED_B


def _get_compiled():
    global _COMPILED
    if _COMPILED is None:
        _COMPILED = _build()
    return _COMPILED


def kernel(queries, keys, values):
    global LAST_RESULT
    queries = np.asarray(queries, dtype=np.float32)
    keys = np.asarray(keys, dtype=np.float32)
    values = np.asarray(values, dtype=np.float32)

    fx, fy, basisdup, mcomp = _constants()
    bf = ml_dtypes.bfloat16

    in_maps = []
    for i in range(NCORES):
        sl = slice(i * D, (i + 1) * D)
        in_maps.append({
            "qT": np.ascontiguousarray(queries[:, :, sl].transpose(0, 2, 1)).astype(bf),
            "kT": np.ascontiguousarray(keys[:, :, sl].transpose(0, 2, 1)).astype(bf),
            "v": np.ascontiguousarray(values[:, :, sl]).astype(bf),
            "fx": fx,
            "fy": fy,
            "basis2": basisdup,
            "mcomp": mcomp,
        })

    kw = {"trace_cores": list(range(NCORES))} if TRACE else {}
    cores = list(range(NCORES))
    if SPLIT:
        nca, ncb = _get_split()
        maps_a = [{k: m[k] for k in ("qT", "kT", "fx", "fy", "mcomp")}
                  for m in in_maps]
        res_a = run_bass_kernel_spmd(nca, maps_a, core_ids=cores, trace=TRACE, **kw)
        ccs_all = np.stack([res_a.results[i]["ccs"] for i in range(NCORES)])
        csum = ccs_all.astype(np.float32).sum(axis=0) * (1.0 / NCORES)
        maps_b = []
        for i in range(NCORES):
            cd_all = (ccs_all[i].astype(np.float32) - csum).astype(bf)  # [128, L]
            cd2 = np.stack([np.concatenate([cd_all[b * NCC:(b + 1) * NCC]] * 2, axis=0)
                            for b in range(B)])                          # [B, 128, L]
            maps_b.append({"cd2": cd2, "v": in_maps[i]["v"],
                           "basis2": in_maps[i]["basis2"]})
        res = run_bass_kernel_spmd(ncb, maps_b, core_ids=cores, trace=TRACE, **kw)
        LAST_RESULT = res
        globals()["LAST_RESULT_A"] = res_a
    else:
        nc = _get_compiled()
        res = run_bass_kernel_spmd(nc, in_maps, core_ids=cores, trace=TRACE, **kw)
        LAST_RESULT = res

    vt_full = np.stack([res.results[i]["out"] for i in range(NCORES)], axis=1)
    # reference: out = transpose(Vt[B,H,d,L], (0,2,1,3)).reshape(B, L, H*d)
    return np.ascontiguousarray(
        vt_full.transpose(0, 2, 1, 3).reshape(B, L, E)
    ).astype(np.float32)


# revision 27
# speedup vs baseline: 1.3040x; 1.0407x over previous
"""AutoCorrelation (Autoformer-style) Bass kernel for one TRN2 chip (8 NeuronCores).

Math: the reference computes, per (b, h):
    corr = irfft(rfft(q, axis=-1) * conj(rfft(k, axis=-1)), n=L)   # [L, L]
    weights = softmax(corr - mean_h(corr), axis=-1)
    Vt = v @ weights                                                # [d, L]
The rfft runs over the d=64 channel axis and the irfft zero-pads 33 bins to
L=2048, so corr[s, :] is a rank-<=66 function of t; the DC term is constant
over t and cancels in softmax.  Collapsing the spectral products
(re*re + im*im -> cos row, im*re - re*im -> sin row) leaves 64 coefficient
rows: the logits are an exact K=64 matmul against a fixed cos/sin basis and
no [L, L] tensor ever exists in DRAM.

Sharding: head h -> core h (both batches per core).  Only the head-mean of
the 64 x 2048 coefficient matrix couples cores.  Default mode (SPLIT=True)
runs two NEFFs: phase A computes coefficients (~40 us), the host sums the
8 cores' 0.5 MB outputs, and phase B (~90 us) does softmax + aggregation —
this is much faster than an on-device AllReduce, which costs 55-60 us of
mostly-fixed latency on this platform (SPLIT=False keeps everything on
device in one NEFF with column-halved AllReduces, ~170 us).

Phase B details: K=64 logits matmuls are row-packed (two concurrent 64-row
PE tiles via base_partition 0/64 of duplicated coefficient/basis tensors);
the delay-aggregation matmuls are column-packed (Vt stored [128, 1024]:
partitions 0-63 hold t 0:1024, partitions 64-127 hold t 1024:2048).  The
softmax exp splits between ScalarE (table exp) and VectorE (custom DVE op
EXP8_ANT: exp(x) ~= (c0 + x(c1 + x c2))^8, valid since logits are bounded
by ~1.5), both with fused free-dim accumulation for the denominator; the
per-row 1/sum folds into the tiny v-tile instead of the weight tile.
"""
import sys
from operator import add as _op_add

sys.path.insert(0, "/opt/trn_rl_repo")

import numpy as np
import ml_dtypes

from concourse import bass, bacc, mybir, tile
from concourse import dve_ops
from concourse.dve_spec import Spec, Src0, C0, C1, C2, Zero, sq, lower
from concourse.dve_uop import DveOpSpec
from concourse.bass_utils import run_bass_kernel_spmd

B, L, E, H, D = 2, 2048, 512, 8, 64
NF = 32          # frequencies 1..32 of the 64-point rfft (DC dropped)
NCOMP = 4 * NF   # 128 raw product rows
NCC = 2 * NF     # 64 compressed coefficient rows (cos, sin)
NCORES = 8
SC = L // 128    # 16 s-chunks of 128 rows
BF16 = mybir.dt.bfloat16
F32 = mybir.dt.float32

# minimax quadratic p(z) for e^z on z = x/8, |x| <= 1.68; exp(x) ~= p(x)^8
EXP_C = (0.99970171, 0.12580122, 0.00795605)

TRACE = False
SPLIT = True
LAST_RESULT = None
LAST_RESULT_A = None

_COMPILED = None
_EXP_OP = None


def _register_exp_op():
    global _EXP_OP
    if _EXP_OP is not None:
        return _EXP_OP
    for o in dve_ops.OPS:
        if o.name == "EXP8_ANT":
            _EXP_OP = o
            return o

    body = sq(sq(sq(C0 + Src0 * (C1 + Src0 * C2))))

    def _ref(in0, in1, c0, c1, c2):
        x = in0.astype(np.float32)
        b = (((c0 + x * (c1 + x * c2)) ** 8)).astype(np.float32)
        return b, b.reshape(b.shape[0], -1).sum(axis=-1, keepdims=True)

    spec = Spec(body=body, accum=_op_add, accum_init=Zero, reference=_ref)
    opcode = dve_ops._CUSTOM_DVE_ROW_BASE + len(dve_ops.OPS)
    dve_ops._SUB_OPCODE_FOR_NAME["EXP8_ANT"] = opcode
    shas = {}
    for ver in ("v3", "v4"):
        shas[ver] = DveOpSpec(
            name="EXP8_ANT", opcode=opcode, uops=lower(spec, ver=ver), rd1_en=False
        ).sha(ver)
    op = dve_ops.DveOp("EXP8_ANT", spec, subdim=False, uops_sha=shas)
    dve_ops.OPS.append(op)
    dve_ops.CUSTOM_DVE_SPECS[op.name] = spec
    _EXP_OP = op
    return op


def _constants():
    c = np.arange(D)
    f = np.arange(1, NF + 1)
    ang = 2 * np.pi * np.outer(c, f) / D
    fcos = np.cos(ang)       # Re X_f   = sum_c q_c cos
    fsin = -np.sin(ang)      # Im X_f   = -sum_c q_c sin
    w = 2.0 / L              # irfft weight for interior bins
    fx = np.concatenate([fcos * w, fsin * w, fsin * w, fcos * w], axis=1)  # [64, 128]
    fy = np.concatenate([fcos, fsin, fcos, fsin], axis=1)                  # [64, 128]
    t = np.arange(L)
    angt = 2 * np.pi * np.outer(f, t) / L
    cosb, sinb = np.cos(angt), np.sin(angt)
    basis64 = np.concatenate([cosb, -sinb], axis=0)                        # [64, 2048]
    basisdup = np.concatenate([basis64, basis64], axis=0)                  # [128, 2048]
    # compression: Ccs[0:32] = P[0:32] + P[32:64]  (re*re + im*im -> cos)
    #              Ccs[32:64] = P[64:96] - P[96:128] (im*re - re*im -> -sin)
    mcomp = np.zeros((NCOMP, NCC), np.float32)
    for m in range(32):
        mcomp[m, m] = 1.0
        mcomp[m + 32, m] = 1.0
        mcomp[m + 64, m + 32] = 1.0
        mcomp[m + 96, m + 32] = -1.0
    bf = ml_dtypes.bfloat16
    return fx.astype(bf), fy.astype(bf), basisdup.astype(bf), mcomp.astype(bf)


def _build():
    exp_op = _register_exp_op()
    nc = bacc.Bacc("TRN2", target_bir_lowering=False, debug=False, num_devices=NCORES)

    qT_d = nc.dram_tensor("qT", [B, D, L], BF16, kind="ExternalInput")
    kT_d = nc.dram_tensor("kT", [B, D, L], BF16, kind="ExternalInput")
    v_d = nc.dram_tensor("v", [B, L, D], BF16, kind="ExternalInput")
    fx_d = nc.dram_tensor("fx", [D, NCOMP], BF16, kind="ExternalInput")
    fy_d = nc.dram_tensor("fy", [D, NCOMP], BF16, kind="ExternalInput")
    basis_d = nc.dram_tensor("basis2", [NCOMP, L], BF16, kind="ExternalInput")
    mcomp_d = nc.dram_tensor("mcomp", [NCOMP, NCC], BF16, kind="ExternalInput")
    out_d = nc.dram_tensor("out", [B, D, L], F32, kind="ExternalOutput")

    rg = [list(range(NCORES))]

    with tile.TileContext(nc) as tc:
        with (
            tc.tile_pool(name="consts", bufs=1) as consts,
            tc.tile_pool(name="qk", bufs=2) as qk_pool,
            tc.tile_pool(name="vv", bufs=2) as v_pool,
            tc.tile_pool(name="xy", bufs=2) as xy_pool,
            tc.tile_pool(name="cf", bufs=2) as cf_pool,
            tc.tile_pool(name="cs", bufs=2) as cs_pool,
            tc.tile_pool(name="cd", bufs=2) as cd_pool,
            tc.tile_pool(name="wts", bufs=6) as w_pool,
            tc.tile_pool(name="small", bufs=12) as s_pool,
            tc.tile_pool(name="outp", bufs=2) as out_pool,
            tc.tile_pool(name="ps_log", bufs=3, space="PSUM") as ps_log,
            tc.tile_pool(name="ps_vt", bufs=1, space="PSUM") as ps_vt,
            tc.tile_pool(name="dram", bufs=1, space="DRAM") as dram,
        ):
            fx_sb = consts.tile([D, NCOMP], BF16)
            fy_sb = consts.tile([D, NCOMP], BF16)
            basis_sb = consts.tile([NCOMP, L], BF16)
            mcomp_sb = consts.tile([NCOMP, NCC], BF16)
            nc.sync.dma_start(out=fx_sb[:], in_=fx_d[:])
            nc.sync.dma_start(out=fy_sb[:], in_=fy_d[:])
            nc.gpsimd.dma_start(out=basis_sb[:], in_=basis_d[:])
            nc.sync.dma_start(out=mcomp_sb[:], in_=mcomp_d[:])

            cc_in_h = [dram.tile([B * NCC, 1024], BF16, name=f"cc_in_h{j}")
                       for j in range(2)]
            cc_out_h = [dram.tile([B * NCC, 1024], BF16, addr_space="Shared",
                                  name=f"cc_out_h{j}") for j in range(2)]

            # Prefetch everything while the coefficient pipeline runs.
            qk_sb = []
            for b in range(B):
                qT_sb = qk_pool.tile([D, L], BF16, tag=f"qT{b}")
                kT_sb = qk_pool.tile([D, L], BF16, tag=f"kT{b}")
                nc.sync.dma_start(out=qT_sb[:], in_=qT_d[b])
                nc.sync.dma_start(out=kT_sb[:], in_=kT_d[b])
                qk_sb.append((qT_sb, kT_sb))
            v_sbs = []
            for b in range(B):
                v_sb = v_pool.tile([128, SC, D], BF16, tag=f"v{b}")
                nc.gpsimd.dma_start(
                    out=v_sb[:], in_=v_d[b].rearrange("(c p) d -> p c d", p=128)
                )
                v_sbs.append(v_sb)

            # ---- Phase 1: compressed coefficients Ccs, b-stacked [128, L] ----
            # Column-halved: the AllReduce for s-columns 0:1024 fires after the
            # first half of the pipeline, and its result is all that the first
            # 8 s-chunks of the main loop need — the second AllReduce hides
            # under main-loop compute.  b0 -> partitions 0:64, b1 -> 64:128.
            ccs_h = [cs_pool.tile([B * NCC, 1024], BF16, tag=f"ccs{j}", name=f"ccs_h{j}")
                     for j in range(2)]
            for j in range(2):  # s-column halves of 1024
                for b in range(B):
                    qT_sb, kT_sb = qk_sb[b]
                    xt2 = xy_pool.tile([NCOMP, 1024], BF16, tag="xt2")
                    yt2 = xy_pool.tile([NCOMP, 1024], BF16, tag="yt2")
                    engs = ("scalar", "vector") if b == 0 else ("vector", "scalar")
                    for src_sb, fmat, dst, cast_eng in (
                        (qT_sb, fx_sb, xt2, engs[0]),
                        (kT_sb, fy_sb, yt2, engs[1]),
                    ):
                        ps = ps_log.tile([NCOMP, 1024], F32, tag="log")
                        for q in range(2):
                            nc.tensor.matmul(
                                ps[:, q * 512:(q + 1) * 512],
                                fmat[:],
                                src_sb[:, j * 1024 + q * 512: j * 1024 + (q + 1) * 512],
                                start=True, stop=True,
                            )
                        if cast_eng == "scalar":
                            nc.scalar.copy(dst[:], ps[:])
                        else:
                            nc.vector.tensor_copy(dst[:], ps[:])

                    cf = cf_pool.tile([NCOMP, 1024], BF16, tag="cfull")
                    nc.vector.tensor_mul(cf[:], xt2[:], yt2[:])

                    ps = ps_log.tile([NCOMP, 1024], F32, tag="log")
                    pcc = ps[b * NCC:(b + 1) * NCC, :]
                    for q in range(2):
                        nc.tensor.matmul(
                            pcc[:, q * 512:(q + 1) * 512],
                            mcomp_sb[:],
                            cf[:, q * 512:(q + 1) * 512],
                            start=True, stop=True,
                        )
                    dst = ccs_h[j][b * NCC:(b + 1) * NCC, :]
                    nc.scalar.copy(dst, pcc)
                    nc.sync.dma_start(
                        out=cc_in_h[j][b * NCC:(b + 1) * NCC, :], in_=dst
                    )
                nc.gpsimd.collective_compute(
                    "AllReduce", mybir.AluOpType.add, replica_groups=rg,
                    ins=[cc_in_h[j][:].opt()], outs=[cc_out_h[j][:].opt()],
                )

            # cd = ccs - mean_h = (csum * -1/8) + ccs, duplicated to both
            # partition halves so K=64 logits matmuls row-pack the PE.
            cd2h = [[None, None], [None, None]]
            for j in range(2):
                csum = cs_pool.tile([B * NCC, 1024], BF16, tag=f"csum{j}",
                                    name=f"csum_h{j}")
                nc.sync.dma_start(out=csum[:], in_=cc_out_h[j][:])
                cda = cs_pool.tile([B * NCC, 1024], BF16, tag=f"cda{j}",
                                   name=f"cd_all{j}")
                nc.vector.scalar_tensor_tensor(
                    cda[:], csum[:], -1.0 / NCORES, ccs_h[j][:],
                    op0=mybir.AluOpType.mult, op1=mybir.AluOpType.add,
                )
                for b in range(B):
                    cdd = cd_pool.tile([2 * NCC, 1024], BF16, tag=f"cd2_{b}{j}",
                                       name=f"cd2_{b}{j}")
                    nc.sync.dma_start(out=cdd[0:NCC, :],
                                      in_=cda[b * NCC:(b + 1) * NCC, :])
                    nc.sync.dma_start(out=cdd[NCC:2 * NCC, :],
                                      in_=cda[b * NCC:(b + 1) * NCC, :])
                    cd2h[b][j] = cdd

            # ---- Phase 2: per-b softmax + delay aggregation ----
            # Vt packed: partitions 0-63 = Vt[:, 0:1024], 64-127 = Vt[:, 1024:2048]
            for b in range(B):
                v_sb = v_sbs[b]
                vt_ps = ps_vt.tile([128, 1024], F32, tag="vt")

                wts_hist = {}
                vts_hist = {}
                sig_hist = {}

                def emit_acc(sc):
                    pwt = wts_hist.pop(sc)
                    pvts = vts_hist.pop(sc)
                    for q in range(2):  # packed pairs: (q, q+2)
                        nc.tensor.matmul(
                            vt_ps[0:D, q * 512:(q + 1) * 512],
                            pvts[:],
                            pwt[0][:, q * 512:(q + 1) * 512],
                            start=(sc == 0), stop=(sc == SC - 1),
                        )
                        nc.tensor.matmul(
                            vt_ps[D:2 * D, q * 512:(q + 1) * 512],
                            pvts[:],
                            pwt[1][:, q * 512:(q + 1) * 512],
                            start=(sc == 0), stop=(sc == SC - 1),
                        )

                def emit_small(sc):
                    sig = sig_hist.pop(sc)
                    sigsum = s_pool.tile([128, 1], F32, tag="sigsum")
                    nc.gpsimd.tensor_add(sigsum[:], sig[:, 0:1], sig[:, 1:2])
                    rcp = s_pool.tile([128, 1], F32, tag="rcp")
                    nc.vector.reciprocal_approx_fast(rcp[:], sigsum[:])
                    vts = s_pool.tile([128, D], BF16, tag="vts")
                    nc.vector.tensor_scalar_mul(vts[:], v_sb[:, sc, :], rcp[:])
                    vts_hist[sc] = vts

                for sc in range(SC):
                    half = cd2h[b][sc // 8]
                    off = (sc % 8) * 128
                    cdt = half[0:NCC, off:off + 128]
                    cdb = half[NCC:2 * NCC, off:off + 128]
                    lg0 = ps_log.tile([128, 1024], F32, tag="log")
                    lg1 = ps_log.tile([128, 1024], F32, tag="log")
                    for q in range(2):
                        # row-packed pair: h2=0 on PE rows 0-63, h2=1 on 64-127
                        nc.tensor.matmul(
                            lg0[:, q * 512:(q + 1) * 512], cdt,
                            basis_sb[0:NCC, q * 512:(q + 1) * 512],
                            start=True, stop=True,
                        )
                        nc.tensor.matmul(
                            lg1[:, q * 512:(q + 1) * 512], cdb,
                            basis_sb[NCC:2 * NCC, 1024 + q * 512: 1024 + (q + 1) * 512],
                            start=True, stop=True,
                        )
                    if sc >= 2:
                        emit_acc(sc - 2)

                    sig = s_pool.tile([128, 2], F32, tag="sig")
                    wt0 = w_pool.tile([128, 1024], BF16, tag="wt")
                    nc.scalar.activation(
                        wt0[:], lg0[:], mybir.ActivationFunctionType.Exp,
                        accum_out=sig[:, 0:1],
                    )
                    wt1 = w_pool.tile([128, 1024], BF16, tag="wt")
                    nc.vector._custom_dve(
                        exp_op, out=wt1[:], in0=lg1[:],
                        s0=EXP_C[0], s1=EXP_C[1], imm2=EXP_C[2],
                        accum_out=sig[:, 1:2],
                    )
                    wts_hist[sc] = (wt0, wt1)
                    sig_hist[sc] = sig
                    if sc >= 1:
                        emit_small(sc - 1)

                emit_small(SC - 1)
                emit_acc(SC - 2)
                emit_acc(SC - 1)

                out_sb = out_pool.tile([128, 1024], F32, tag="out")
                nc.vector.tensor_copy(out_sb[:], vt_ps[:])
                nc.sync.dma_start(out=out_d[b][:, 0:1024], in_=out_sb[0:D, :])
                nc.sync.dma_start(out=out_d[b][:, 1024:2048], in_=out_sb[D:2 * D, :])

    nc.compile()
    return nc



_COMPILED_A = None
_COMPILED_B = None


def _build_split_a():
    """NEFF A: coefficient pipeline only.  Outputs b-stacked Ccs [128, L]."""
    _register_exp_op()
    nc = bacc.Bacc("TRN2", target_bir_lowering=False, debug=False, num_devices=NCORES)
    qk_d = nc.dram_tensor("qkT", [B, 2 * D, L], BF16, kind="ExternalInput")
    fxy_d = nc.dram_tensor("fxy", [2 * D, NCOMP], BF16, kind="ExternalInput")
    mcomp_d = nc.dram_tensor("mcomp", [NCOMP, NCC], BF16, kind="ExternalInput")
    ccs_d = nc.dram_tensor("ccs", [B * NCC, L], BF16, kind="ExternalOutput")

    with tile.TileContext(nc) as tc:
        with (
            tc.tile_pool(name="consts", bufs=1) as consts,
            tc.tile_pool(name="qk", bufs=2) as qk_pool,
            tc.tile_pool(name="xy", bufs=4) as xy_pool,
            tc.tile_pool(name="cf", bufs=4) as cf_pool,
            tc.tile_pool(name="cs", bufs=4) as cs_pool,
            tc.tile_pool(name="ps", bufs=4, space="PSUM") as ps_pool,
        ):
            fxy_sb = consts.tile([2 * D, NCOMP], BF16)
            mcomp_sb = consts.tile([NCOMP, NCC], BF16)
            nc.sync.dma_start(out=fxy_sb[:], in_=fxy_d[:])
            nc.sync.dma_start(out=mcomp_sb[:], in_=mcomp_d[:])
            qk_sb = []
            for b in range(B):
                qk_t = qk_pool.tile([2 * D, L], BF16, tag=f"qk{b}")
                nc.sync.dma_start(out=qk_t[:], in_=qk_d[b])
                qk_sb.append(qk_t)

            for b in range(B):
                qk_t = qk_sb[b]
                for j in range(2):
                    xt2 = xy_pool.tile([NCOMP, 1024], BF16, tag="xt2")
                    yt2 = xy_pool.tile([NCOMP, 1024], BF16, tag="yt2")
                    psx = ps_pool.tile([NCOMP, 1024], F32, tag="log")
                    psy = ps_pool.tile([NCOMP, 1024], F32, tag="log")
                    for q in range(2):
                        cols = slice(j * 1024 + q * 512, j * 1024 + (q + 1) * 512)
                        # row-packed pair: q-spectrum on PE rows 0-63,
                        # k-spectrum on rows 64-127, concurrent
                        nc.tensor.matmul(
                            psx[:, q * 512:(q + 1) * 512],
                            fxy_sb[0:D, :], qk_t[0:D, cols],
                            start=True, stop=True,
                        )
                        nc.tensor.matmul(
                            psy[:, q * 512:(q + 1) * 512],
                            fxy_sb[D:2 * D, :], qk_t[D:2 * D, cols],
                            start=True, stop=True,
                        )
                    eng0 = "scalar" if b == 0 else "vector"
                    if eng0 == "scalar":
                        nc.scalar.copy(xt2[:], psx[:])
                        nc.vector.tensor_copy(yt2[:], psy[:])
                    else:
                        nc.vector.tensor_copy(xt2[:], psx[:])
                        nc.scalar.copy(yt2[:], psy[:])
                    cf = cf_pool.tile([NCOMP, 1024], BF16, tag="cfull")
                    nc.vector.tensor_mul(cf[:], xt2[:], yt2[:])
                    ps = ps_pool.tile([NCOMP, 1024], F32, tag="log")
                    pcc = ps[b * NCC:(b + 1) * NCC, :]
                    for q in range(2):
                        nc.tensor.matmul(
                            pcc[:, q * 512:(q + 1) * 512],
                            mcomp_sb[:],
                            cf[:, q * 512:(q + 1) * 512],
                            start=True, stop=True,
                        )
                    ccs = cs_pool.tile([NCC, 1024], BF16, tag="ccs")
                    nc.scalar.copy(ccs[:], pcc)
                    nc.sync.dma_start(
                        out=ccs_d[b * NCC:(b + 1) * NCC, j * 1024:(j + 1) * 1024],
                        in_=ccs[:],
                    )
    nc.compile()
    return nc


def _build_split_b():
    """NEFF B: softmax + delay aggregation from host-reduced coefficients."""
    exp_op = _register_exp_op()
    nc = bacc.Bacc("TRN2", target_bir_lowering=False, debug=False, num_devices=NCORES)
    cd_d = nc.dram_tensor("cd2", [B, 2 * NCC, L], BF16, kind="ExternalInput")
    v_d = nc.dram_tensor("v", [B, L, D], BF16, kind="ExternalInput")
    basis_d = nc.dram_tensor("basis2", [NCOMP, L], BF16, kind="ExternalInput")
    out_d = nc.dram_tensor("out", [B, D, L], F32, kind="ExternalOutput")

    with tile.TileContext(nc) as tc:
        with (
            tc.tile_pool(name="consts", bufs=1) as consts,
            tc.tile_pool(name="vv", bufs=2) as v_pool,
            tc.tile_pool(name="cd", bufs=2) as cd_pool,
            tc.tile_pool(name="wts", bufs=10) as w_pool,
            tc.tile_pool(name="small", bufs=12) as s_pool,
            tc.tile_pool(name="outp", bufs=2) as out_pool,
            tc.tile_pool(name="ps_log", bufs=3, space="PSUM") as ps_log,
            tc.tile_pool(name="ps_vt", bufs=1, space="PSUM") as ps_vt,
        ):
            basis_sb = consts.tile([NCOMP, L], BF16)
            nc.sync.dma_start(out=basis_sb[:], in_=basis_d[:])
            cd_sbs = []
            v_sbs = []
            for b in range(B):
                halves = []
                for j in range(2):
                    cdd = cd_pool.tile([2 * NCC, 1024], BF16, tag=f"cd{b}{j}")
                    nc.sync.dma_start(out=cdd[:], in_=cd_d[b][:, j * 1024:(j + 1) * 1024])
                    halves.append(cdd)
                cd_sbs.append(halves)
                v_sb = v_pool.tile([128, SC, D], BF16, tag=f"v{b}")
                nc.gpsimd.dma_start(
                    out=v_sb[:], in_=v_d[b].rearrange("(c p) d -> p c d", p=128)
                )
                v_sbs.append(v_sb)

            for b in range(B):
                v_sb = v_sbs[b]
                vt_ps = ps_vt.tile([128, 1024], F32, tag="vt")
                wts_hist = {}
                vts_hist = {}
                sig_hist = {}

                def emit_acc(sc):
                    pwt = wts_hist.pop(sc)
                    pvts = vts_hist.pop(sc)
                    for q in range(2):
                        nc.tensor.matmul(
                            vt_ps[0:D, q * 512:(q + 1) * 512],
                            pvts[:],
                            pwt[0][:, q * 512:(q + 1) * 512],
                            start=(sc == 0), stop=(sc == SC - 1),
                        )
                        nc.tensor.matmul(
                            vt_ps[D:2 * D, q * 512:(q + 1) * 512],
                            pvts[:],
                            pwt[1][:, q * 512:(q + 1) * 512],
                            start=(sc == 0), stop=(sc == SC - 1),
                        )

                def emit_small(sc):
                    sig = sig_hist.pop(sc)
                    sigsum = s_pool.tile([128, 1], F32, tag="sigsum")
                    nc.gpsimd.tensor_add(sigsum[:], sig[:, 0:1], sig[:, 1:2])
                    rcp = s_pool.tile([128, 1], F32, tag="rcp")
                    nc.vector.reciprocal_approx_fast(rcp[:], sigsum[:])
                    vts = s_pool.tile([128, D], BF16, tag="vts")
                    nc.gpsimd.tensor_scalar_mul(vts[:], v_sb[:, sc, :], rcp[:])
                    vts_hist[sc] = vts

                for sc in range(SC):
                    half = cd_sbs[b][sc // 8]
                    off = (sc % 8) * 128
                    cdt = half[0:NCC, off:off + 128]
                    cdb = half[NCC:2 * NCC, off:off + 128]
                    lg0 = ps_log.tile([128, 1024], F32, tag="log")
                    lg1 = ps_log.tile([128, 1024], F32, tag="log")
                    for q in range(2):
                        nc.tensor.matmul(
                            lg0[:, q * 512:(q + 1) * 512], cdt,
                            basis_sb[0:NCC, q * 512:(q + 1) * 512],
                            start=True, stop=True,
                        )
                        nc.tensor.matmul(
                            lg1[:, q * 512:(q + 1) * 512], cdb,
                            basis_sb[NCC:2 * NCC, 1024 + q * 512: 1024 + (q + 1) * 512],
                            start=True, stop=True,
                        )
                    if sc >= 2:
                        emit_acc(sc - 2)
                    sig = s_pool.tile([128, 2], F32, tag="sig")
                    wt0 = w_pool.tile([128, 1024], BF16, tag="wt")
                    nc.scalar.activation(
                        wt0[:], lg0[:], mybir.ActivationFunctionType.Exp,
                        accum_out=sig[:, 0:1],
                    )
                    wt1 = w_pool.tile([128, 1024], BF16, tag="wt")
                    nc.vector._custom_dve(
                        exp_op, out=wt1[:], in0=lg1[:],
                        s0=EXP_C[0], s1=EXP_C[1], imm2=EXP_C[2],
                        accum_out=sig[:, 1:2],
                    )
                    wts_hist[sc] = (wt0, wt1)
                    sig_hist[sc] = sig
                    if sc >= 1:
                        emit_small(sc - 1)

                emit_small(SC - 1)
                emit_acc(SC - 2)
                emit_acc(SC - 1)

                out_sb = out_pool.tile([128, 1024], F32, tag="out")
                nc.vector.tensor_copy(out_sb[:], vt_ps[:])
                nc.sync.dma_start(out=out_d[b][:, 0:1024], in_=out_sb[0:D, :])
                nc.sync.dma_start(out=out_d[b][:, 1024:2048], in_=out_sb[D:2 * D, :])
    nc.compile()
    return nc


def _get_split():
    global _COMPILED_A, _COMPILED_B
    if _COMPILED_A is None:
        _COMPILED_A = _build_split_a()
        _COMPILED_B = _build_split_b()
    return _COMPILED_A, _COMPILED_B


def _get_compiled():
    global _COMPILED
    if _COMPILED is None:
        _COMPILED = _build()
    return _COMPILED


def kernel(queries, keys, values):
    global LAST_RESULT
    queries = np.asarray(queries, dtype=np.float32)
    keys = np.asarray(keys, dtype=np.float32)
    values = np.asarray(values, dtype=np.float32)

    fx, fy, basisdup, mcomp = _constants()
    bf = ml_dtypes.bfloat16

    in_maps = []
    for i in range(NCORES):
        sl = slice(i * D, (i + 1) * D)
        qT_i = np.ascontiguousarray(queries[:, :, sl].transpose(0, 2, 1)).astype(bf)
        kT_i = np.ascontiguousarray(keys[:, :, sl].transpose(0, 2, 1)).astype(bf)
        in_maps.append({
            "qT": qT_i,
            "kT": kT_i,
            "qkT": np.concatenate([qT_i, kT_i], axis=1),
            "v": np.ascontiguousarray(values[:, :, sl]).astype(bf),
            "fx": fx,
            "fy": fy,
            "fxy": np.concatenate([fx, fy], axis=0),
            "basis2": basisdup,
            "mcomp": mcomp,
        })

    kw = {"trace_cores": list(range(NCORES))} if TRACE else {}
    cores = list(range(NCORES))
    if SPLIT:
        nca, ncb = _get_split()
        maps_a = [{k: m[k] for k in ("qkT", "fxy", "mcomp")}
                  for m in in_maps]
        res_a = run_bass_kernel_spmd(nca, maps_a, core_ids=cores, trace=TRACE, **kw)
        ccs_all = np.stack([res_a.results[i]["ccs"] for i in range(NCORES)])
        csum = ccs_all.astype(np.float32).sum(axis=0) * (1.0 / NCORES)
        maps_b = []
        for i in range(NCORES):
            cd_all = (ccs_all[i].astype(np.float32) - csum).astype(bf)  # [128, L]
            cd2 = np.stack([np.concatenate([cd_all[b * NCC:(b + 1) * NCC]] * 2, axis=0)
                            for b in range(B)])                          # [B, 128, L]
            maps_b.append({"cd2": cd2, "v": in_maps[i]["v"],
                           "basis2": in_maps[i]["basis2"]})
        res = run_bass_kernel_spmd(ncb, maps_b, core_ids=cores, trace=TRACE, **kw)
        LAST_RESULT = res
        globals()["LAST_RESULT_A"] = res_a
    else:
        nc = _get_compiled()
        res = run_bass_kernel_spmd(nc, in_maps, core_ids=cores, trace=TRACE, **kw)
        LAST_RESULT = res

    vt_full = np.stack([res.results[i]["out"] for i in range(NCORES)], axis=1)
    # reference: out = transpose(Vt[B,H,d,L], (0,2,1,3)).reshape(B, L, H*d)
    return np.ascontiguousarray(
        vt_full.transpose(0, 2, 1, 3).reshape(B, L, E)
    ).astype(np.float32)
